# revision 10
# baseline (speedup 1.0000x reference)
"""ChunkEmbedding Trainium2 kernel.

Computation (see reference):
  chunk[n, :] = sum_l (w[n,l]/sum_l w[n,l]) * emb_table[input_ids[n,l], :]
  then scatter chunk rows into [B, T, D] at (map_ids[n], pos[n]+1), add
  CLS/SEP rows and build the mask.

Strategy:
  - Data-parallel over chunks: 4096 chunks -> 512 per core on 8 cores.
  - Per core: for each tile of 128 chunks, for each token l in 0..63,
    indirect-DMA-gather emb rows [128, 768] (partition = chunk) and
    fused multiply-accumulate on the vector engine:
        acc = x * w[:, l] + acc        (w is a per-partition scalar)
  - Weight normalization, final scatter into the padded [B, T, D] output,
    CLS/SEP and mask are trivial O(input/output-size) host-side prep on
    numpy (part of shard/unshard).
"""

import os

import numpy as np

N_CORES = 8
N_CHUNKS = 4096
CHUNK_LEN = 64
DIM = 768
VOCAB = 30522
CPC = N_CHUNKS // N_CORES  # chunks per core = 512
P = 128
N_TILES = CPC // P  # 4
CLS_IDX, SEP_IDX = 101, 102

# Pool depth for in-flight gather tiles (each is [128, G, 768] f32 = G*3KB/partition)
GATHER_BUFS = int(os.environ.get("CHUNKEMB_GATHER_BUFS", "4"))

_cache = {}


def _ensure_axon_ntff_hook():
    """The image's antenv package lacks axon_hooks, so trn_boot's NTFF
    profile hook registration silently degraded. Synthesize the module and
    register the ctypes-based hook so run_bass_kernel_spmd(trace=True) can
    capture NTFF profiles through the axon terminal."""
    import sys
    import types

    try:
        from antenv.axon_hooks import get_axon_ntff_profile_hook  # noqa: F401

        return True
    except ImportError:
        pass
    try:
        import antenv
        from trn_agent_boot.trn_boot import _ntff_profile_via_ctypes

        hook = _ntff_profile_via_ctypes("/opt/axon/libaxon_pjrt.so")
        if hook is None:
            return False
        mod = types.ModuleType("antenv.axon_hooks")
        mod._hook = hook
        mod.set_axon_ntff_profile_hook = lambda h: setattr(mod, "_hook", h)
        mod.get_axon_ntff_profile_hook = lambda: mod._hook
        sys.modules["antenv.axon_hooks"] = mod
        antenv.axon_hooks = mod
        return True
    except Exception:
        return False


# tokens gathered per dma_gather instruction (num_idxs = G*128)
G = int(os.environ.get("CHUNKEMB_G", "8"))
N_GROUPS = CHUNK_LEN // G
# "dve": fused multiply-accumulate chain on VectorE
# "pe":  diagonal-weight matmuls accumulating in PSUM on TensorE
IMPL = os.environ.get("CHUNKEMB_IMPL", "dve")
N_QUEUES = int(os.environ.get("CHUNKEMB_QUEUES", "1"))


def _build_program():
    import concourse.bacc as bacc
    import concourse.mybir as mybir
    import concourse.tile as tile

    nc = bacc.Bacc(
        "TRN2",
        target_bir_lowering=False,
        debug=False,
        num_devices=N_CORES,
        num_swdge_queues=N_QUEUES,
        dynamic_dma_scratch_size=int(
            os.environ.get("CHUNKEMB_SCRATCH", "65536")
        ),
    )
    emb = nc.dram_tensor(
        "emb", [VOCAB, DIM], mybir.dt.float32, kind="ExternalInput"
    ).ap()
    # int16 gather indices, pre-wrapped in 16 partitions (replicated to 128)
    # per dma_gather's layout: flat idx j lives at [j%16, j//16].
    idxs16 = nc.dram_tensor(
        "idxs16",
        [N_TILES, N_GROUPS, P, (G * P) // 16],
        mybir.dt.int16,
        kind="ExternalInput",
    ).ap()
    wn = nc.dram_tensor(
        "wn", [CPC, CHUNK_LEN], mybir.dt.float32, kind="ExternalInput"
    ).ap()
    out = nc.dram_tensor(
        "out", [CPC, DIM], mybir.dt.float32, kind="ExternalOutput"
    ).ap()

    num_idxs = G * P

    from concourse.masks import make_identity

    with tile.TileContext(nc) as tc:
        with (
            tc.tile_pool(name="xpool", bufs=GATHER_BUFS) as xpool,
            tc.tile_pool(name="idxp", bufs=4) as idxpool,
            tc.tile_pool(name="meta", bufs=2) as metapool,
            tc.tile_pool(name="accp", bufs=2) as accpool,
            tc.tile_pool(name="ident", bufs=1) as identpool,
            tc.tile_pool(name="diagp", bufs=4) as diagpool,
            tc.tile_pool(name="psump", bufs=2, space="PSUM") as psumpool,
        ):
            ident = None
            if IMPL == "pe":
                ident = identpool.tile([P, P], mybir.dt.float32, tag="ident")
                make_identity(nc, ident[:])
            gather_i = 0
            for t in range(N_TILES):
                wn_t = metapool.tile([P, CHUNK_LEN], mybir.dt.float32, tag="wn")
                nc.sync.dma_start(out=wn_t[:], in_=wn[t * P : (t + 1) * P, :])
                if IMPL == "pe":
                    psum_a = psumpool.tile([P, 512], mybir.dt.float32, tag="pa")
                    psum_b = psumpool.tile([P, 256], mybir.dt.float32, tag="pb")
                    acc = accpool.tile([P, DIM], mybir.dt.float32, tag="acc")
                else:
                    acc = accpool.tile([P, DIM], mybir.dt.float32, tag="acc")
                for gi in range(N_GROUPS):
                    idx_t = idxpool.tile(
                        [P, (G * P) // 16], mybir.dt.int16, tag="idx"
                    )
                    nc.sync.dma_start(out=idx_t[:], in_=idxs16[t, gi])
                    x = xpool.tile([P, G, DIM], mybir.dt.float32, tag="x")
                    nc.gpsimd.dma_gather(
                        x[:],
                        emb[:],
                        idx_t[:],
                        num_idxs,
                        num_idxs,
                        DIM,
                        queue_num=gather_i % N_QUEUES,
                    )
                    gather_i += 1
                    for g in range(G):
                        l = gi * G + g
                        if IMPL == "pe":
                            diag = diagpool.tile(
                                [P, P], mybir.dt.float32, tag="diag"
                            )
                            nc.vector.tensor_scalar_mul(
                                diag[:], ident[:], wn_t[:, l : l + 1]
                            )
                            nc.tensor.matmul(
                                out=psum_a[:],
                                lhsT=diag[:],
                                rhs=x[:, g, 0:512],
                                start=(l == 0),
                                stop=(l == CHUNK_LEN - 1),
                            )
                            nc.tensor.matmul(
                                out=psum_b[:],
                                lhsT=diag[:],
                                rhs=x[:, g, 512:DIM],
                                start=(l == 0),
                                stop=(l == CHUNK_LEN - 1),
                            )
                        elif l == 0:
                            nc.vector.tensor_scalar_mul(
                                acc[:], x[:, g, :], wn_t[:, 0:1]
                            )
                        else:
                            nc.vector.scalar_tensor_tensor(
                                out=acc[:],
                                in0=x[:, g, :],
                                scalar=wn_t[:, l : l + 1],
                                in1=acc[:],
                                op0=mybir.AluOpType.mult,
                                op1=mybir.AluOpType.add,
                            )
                if IMPL == "pe":
                    nc.scalar.copy(acc[:, 0:512], psum_a[:])
                    nc.scalar.copy(acc[:, 512:DIM], psum_b[:])
                nc.sync.dma_start(out=out[t * P : (t + 1) * P, :], in_=acc[:])
    nc.compile()
    return nc


def _pack_idxs(ids_core):
    """ids_core [CPC, CHUNK_LEN] int -> [N_TILES, N_GROUPS, 128, G*128//16]
    int16 in dma_gather's wrapped layout: per (tile, group), flat position
    i = g*128 + c holds ids[tile*128+c, gi*G+g]; wrapped[p, s] = flat[s*16+p],
    replicated across the 8 groups of 16 partitions."""
    a = ids_core.reshape(N_TILES, P, N_GROUPS, G)  # t, c, gi, g
    a = a.transpose(0, 2, 3, 1)  # t, gi, g, c  -> flat (g*128+c)
    flat = a.reshape(N_TILES, N_GROUPS, G * P)
    wrapped = flat.reshape(N_TILES, N_GROUPS, (G * P) // 16, 16).transpose(
        0, 1, 3, 2
    )  # [t, gi, 16, s]
    rep = np.tile(wrapped, (1, 1, 8, 1))  # replicate to 128 partitions
    return np.ascontiguousarray(rep.astype(np.int16))


def _get_program():
    if "nc" not in _cache:
        _cache["nc"] = _build_program()
    return _cache["nc"]


def kernel(input_ids, kp_token_weights, map_ids, emb_table, batch_size, max_map_len):
    from concourse.bass_utils import run_bass_kernel_spmd

    ids = np.ascontiguousarray(np.asarray(input_ids, dtype=np.int32))
    w = np.asarray(kp_token_weights, dtype=np.float32)
    wn = np.ascontiguousarray(w / w.sum(axis=1, keepdims=True))
    emb = np.ascontiguousarray(np.asarray(emb_table, dtype=np.float32))
    assert ids.shape == (N_CHUNKS, CHUNK_LEN) and emb.shape == (VOCAB, DIM)

    nc = _get_program()
    in_maps = [
        {
            "emb": emb,
            "idxs16": _pack_idxs(ids[k * CPC : (k + 1) * CPC]),
            "wn": wn[k * CPC : (k + 1) * CPC],
        }
        for k in range(N_CORES)
    ]
    trace = bool(int(os.environ.get("CHUNKEMB_TRACE", "0")))
    if trace:
        trace = _ensure_axon_ntff_hook()
    res = run_bass_kernel_spmd(
        nc,
        in_maps,
        core_ids=list(range(N_CORES)),
        trace=trace,
        trace_cores=list(range(N_CORES)) if trace else None,
    )
    _cache["last_results"] = res
    chunk = np.concatenate([r["out"] for r in res.results], axis=0)  # [4096, 768]

    # Host-side unshard: scatter chunk rows into the padded output.
    mi = np.asarray(map_ids, dtype=np.int64)
    B = int(batch_size)
    T = int(max_map_len) + 2
    counts = np.bincount(mi, minlength=B)
    offsets = np.cumsum(counts) - counts
    pos = np.arange(mi.shape[0], dtype=np.int64) - offsets[mi]
    ret = np.zeros((B, T, DIM), dtype=np.float32)
    ret[mi, pos + 1] = chunk
    ret[:, 0, :] = emb[CLS_IDX]
    ret[np.arange(B), counts + 1] = emb[SEP_IDX]
    mask = np.zeros((B, T), dtype=np.float32)
    mask[mi, pos + 1] = 1.0
    mask[:, 0] = 1.0
    mask[np.arange(B), counts + 1] = 1.0
    return ret, mask


# revision 14
# speedup vs baseline: 1.0195x; 1.0195x over previous
"""ChunkEmbedding Trainium2 kernel.

Computation (see reference):
  chunk[n, :] = sum_l (w[n,l]/sum_l w[n,l]) * emb_table[input_ids[n,l], :]
  then scatter chunk rows into [B, T, D] at (map_ids[n], pos[n]+1), add
  CLS/SEP rows and build the mask.

Strategy:
  - Data-parallel over chunks: 4096 chunks -> 512 per core on 8 cores.
  - Per core: for each tile of 128 chunks, for each token l in 0..63,
    indirect-DMA-gather emb rows [128, 768] (partition = chunk) and
    fused multiply-accumulate on the vector engine:
        acc = x * w[:, l] + acc        (w is a per-partition scalar)
  - Weight normalization, final scatter into the padded [B, T, D] output,
    CLS/SEP and mask are trivial O(input/output-size) host-side prep on
    numpy (part of shard/unshard).
"""

import os

import numpy as np

N_CORES = 8
N_CHUNKS = 4096
CHUNK_LEN = 64
DIM = 768
VOCAB = 30522
CPC = N_CHUNKS // N_CORES  # chunks per core = 512
P = 128
N_TILES = CPC // P  # 4
CLS_IDX, SEP_IDX = 101, 102

# Pool depth for in-flight gather tiles (each is [128, G, 768] f32 = G*3KB/partition)
GATHER_BUFS = int(os.environ.get("CHUNKEMB_GATHER_BUFS", "4"))

_cache = {}


def _ensure_axon_ntff_hook():
    """The image's antenv package lacks axon_hooks, so trn_boot's NTFF
    profile hook registration silently degraded. Synthesize the module and
    register the ctypes-based hook so run_bass_kernel_spmd(trace=True) can
    capture NTFF profiles through the axon terminal."""
    import sys
    import types

    try:
        from antenv.axon_hooks import get_axon_ntff_profile_hook  # noqa: F401

        return True
    except ImportError:
        pass
    try:
        import antenv
        from trn_agent_boot.trn_boot import _ntff_profile_via_ctypes

        hook = _ntff_profile_via_ctypes("/opt/axon/libaxon_pjrt.so")
        if hook is None:
            return False
        mod = types.ModuleType("antenv.axon_hooks")
        mod._hook = hook
        mod.set_axon_ntff_profile_hook = lambda h: setattr(mod, "_hook", h)
        mod.get_axon_ntff_profile_hook = lambda: mod._hook
        sys.modules["antenv.axon_hooks"] = mod
        antenv.axon_hooks = mod
        return True
    except Exception:
        return False


# tokens gathered per dma_gather instruction (num_idxs = G*128)
G = int(os.environ.get("CHUNKEMB_G", "8"))
N_GROUPS = CHUNK_LEN // G
# "dve":   fused multiply-accumulate chain on VectorE
# "pe":    diagonal-weight matmuls accumulating in PSUM on TensorE
# "split": even tokens on VectorE chain, odd tokens on TensorE (diags via ScalarE)
IMPL = os.environ.get("CHUNKEMB_IMPL", "dve")
N_QUEUES = int(os.environ.get("CHUNKEMB_QUEUES", "1"))


def _build_program():
    import concourse.bacc as bacc
    import concourse.mybir as mybir
    import concourse.tile as tile

    nc = bacc.Bacc(
        "TRN2",
        target_bir_lowering=False,
        debug=False,
        num_devices=N_CORES,
        num_swdge_queues=N_QUEUES,
        dynamic_dma_scratch_size=int(
            os.environ.get("CHUNKEMB_SCRATCH", "16384")
        ),
    )
    emb = nc.dram_tensor(
        "emb", [VOCAB, DIM], mybir.dt.float32, kind="ExternalInput"
    ).ap()
    # int16 gather indices, pre-wrapped in 16 partitions (replicated to 128)
    # per dma_gather's layout: flat idx j lives at [j%16, j//16].
    idxs16 = nc.dram_tensor(
        "idxs16",
        [N_TILES, N_GROUPS, P, (G * P) // 16],
        mybir.dt.int16,
        kind="ExternalInput",
    ).ap()
    wn = nc.dram_tensor(
        "wn", [CPC, CHUNK_LEN], mybir.dt.float32, kind="ExternalInput"
    ).ap()
    out = nc.dram_tensor(
        "out", [CPC, DIM], mybir.dt.float32, kind="ExternalOutput"
    ).ap()

    num_idxs = G * P

    from concourse.masks import make_identity

    with tile.TileContext(nc) as tc:
        with (
            tc.tile_pool(name="xpool", bufs=GATHER_BUFS) as xpool,
            tc.tile_pool(name="idxp", bufs=4) as idxpool,
            tc.tile_pool(name="meta", bufs=2) as metapool,
            tc.tile_pool(name="accp", bufs=2) as accpool,
            tc.tile_pool(name="ident", bufs=1) as identpool,
            tc.tile_pool(name="diagp", bufs=4) as diagpool,
            tc.tile_pool(name="psump", bufs=2, space="PSUM") as psumpool,
        ):
            ident = None
            if IMPL in ("pe", "split"):
                ident = identpool.tile([P, P], mybir.dt.float32, tag="ident")
                make_identity(nc, ident[:])
            gather_i = 0
            for t in range(N_TILES):
                wn_t = metapool.tile([P, CHUNK_LEN], mybir.dt.float32, tag="wn")
                nc.sync.dma_start(out=wn_t[:], in_=wn[t * P : (t + 1) * P, :])
                psum_a = psum_b = None
                if IMPL in ("pe", "split"):
                    psum_a = psumpool.tile([P, 512], mybir.dt.float32, tag="pa")
                    psum_b = psumpool.tile([P, 256], mybir.dt.float32, tag="pb")
                acc = accpool.tile([P, DIM], mybir.dt.float32, tag="acc")
                for gi in range(N_GROUPS):
                    idx_t = idxpool.tile(
                        [P, (G * P) // 16], mybir.dt.int16, tag="idx"
                    )
                    nc.sync.dma_start(out=idx_t[:], in_=idxs16[t, gi])
                    x = xpool.tile([P, G, DIM], mybir.dt.float32, tag="x")
                    nc.gpsimd.dma_gather(
                        x[:],
                        emb[:],
                        idx_t[:],
                        num_idxs,
                        num_idxs,
                        DIM,
                        queue_num=gather_i % N_QUEUES,
                    )
                    gather_i += 1
                    for g in range(G):
                        l = gi * G + g
                        on_pe = IMPL == "pe" or (IMPL == "split" and l % 2 == 1)
                        if on_pe:
                            diag = diagpool.tile(
                                [P, P], mybir.dt.float32, tag="diag"
                            )
                            if IMPL == "split":
                                # build diag on the otherwise-idle ScalarE
                                nc.scalar.activation(
                                    diag[:],
                                    ident[:],
                                    mybir.ActivationFunctionType.Copy,
                                    scale=wn_t[:, l : l + 1],
                                )
                                first = l == 1
                            else:
                                nc.vector.tensor_scalar_mul(
                                    diag[:], ident[:], wn_t[:, l : l + 1]
                                )
                                first = l == 0
                            nc.tensor.matmul(
                                out=psum_a[:],
                                lhsT=diag[:],
                                rhs=x[:, g, 0:512],
                                start=first,
                                stop=(l == CHUNK_LEN - 1),
                            )
                            nc.tensor.matmul(
                                out=psum_b[:],
                                lhsT=diag[:],
                                rhs=x[:, g, 512:DIM],
                                start=first,
                                stop=(l == CHUNK_LEN - 1),
                            )
                        elif l == 0:
                            nc.vector.tensor_scalar_mul(
                                acc[:], x[:, g, :], wn_t[:, 0:1]
                            )
                        else:
                            nc.vector.scalar_tensor_tensor(
                                out=acc[:],
                                in0=x[:, g, :],
                                scalar=wn_t[:, l : l + 1],
                                in1=acc[:],
                                op0=mybir.AluOpType.mult,
                                op1=mybir.AluOpType.add,
                            )
                if IMPL == "pe":
                    nc.scalar.copy(acc[:, 0:512], psum_a[:])
                    nc.scalar.copy(acc[:, 512:DIM], psum_b[:])
                elif IMPL == "split":
                    nc.vector.tensor_add(acc[:, 0:512], acc[:, 0:512], psum_a[:])
                    nc.vector.tensor_add(
                        acc[:, 512:DIM], acc[:, 512:DIM], psum_b[:]
                    )
                nc.sync.dma_start(out=out[t * P : (t + 1) * P, :], in_=acc[:])
    nc.compile()
    return nc


def _pack_idxs(ids_core):
    """ids_core [CPC, CHUNK_LEN] int -> [N_TILES, N_GROUPS, 128, G*128//16]
    int16 in dma_gather's wrapped layout: per (tile, group), flat position
    i = g*128 + c holds ids[tile*128+c, gi*G+g]; wrapped[p, s] = flat[s*16+p],
    replicated across the 8 groups of 16 partitions."""
    a = ids_core.reshape(N_TILES, P, N_GROUPS, G)  # t, c, gi, g
    a = a.transpose(0, 2, 3, 1)  # t, gi, g, c  -> flat (g*128+c)
    flat = a.reshape(N_TILES, N_GROUPS, G * P)
    wrapped = flat.reshape(N_TILES, N_GROUPS, (G * P) // 16, 16).transpose(
        0, 1, 3, 2
    )  # [t, gi, 16, s]
    rep = np.tile(wrapped, (1, 1, 8, 1))  # replicate to 128 partitions
    return np.ascontiguousarray(rep.astype(np.int16))


def _get_program():
    if "nc" not in _cache:
        _cache["nc"] = _build_program()
    return _cache["nc"]


def kernel(input_ids, kp_token_weights, map_ids, emb_table, batch_size, max_map_len):
    from concourse.bass_utils import run_bass_kernel_spmd

    ids = np.ascontiguousarray(np.asarray(input_ids, dtype=np.int32))
    w = np.asarray(kp_token_weights, dtype=np.float32)
    wn = np.ascontiguousarray(w / w.sum(axis=1, keepdims=True))
    emb = np.ascontiguousarray(np.asarray(emb_table, dtype=np.float32))
    assert ids.shape == (N_CHUNKS, CHUNK_LEN) and emb.shape == (VOCAB, DIM)

    nc = _get_program()
    in_maps = [
        {
            "emb": emb,
            "idxs16": _pack_idxs(ids[k * CPC : (k + 1) * CPC]),
            "wn": wn[k * CPC : (k + 1) * CPC],
        }
        for k in range(N_CORES)
    ]
    trace = bool(int(os.environ.get("CHUNKEMB_TRACE", "0")))
    if trace:
        trace = _ensure_axon_ntff_hook()
    res = run_bass_kernel_spmd(
        nc,
        in_maps,
        core_ids=list(range(N_CORES)),
        trace=trace,
        trace_cores=list(range(N_CORES)) if trace else None,
    )
    _cache["last_results"] = res
    chunk = np.concatenate([r["out"] for r in res.results], axis=0)  # [4096, 768]

    # Host-side unshard: scatter chunk rows into the padded output.
    mi = np.asarray(map_ids, dtype=np.int64)
    B = int(batch_size)
    T = int(max_map_len) + 2
    counts = np.bincount(mi, minlength=B)
    offsets = np.cumsum(counts) - counts
    pos = np.arange(mi.shape[0], dtype=np.int64) - offsets[mi]
    ret = np.zeros((B, T, DIM), dtype=np.float32)
    ret[mi, pos + 1] = chunk
    ret[:, 0, :] = emb[CLS_IDX]
    ret[np.arange(B), counts + 1] = emb[SEP_IDX]
    mask = np.zeros((B, T), dtype=np.float32)
    mask[mi, pos + 1] = 1.0
    mask[:, 0] = 1.0
    mask[np.arange(B), counts + 1] = 1.0
    return ret, mask


# revision 18
# speedup vs baseline: 1.0750x; 1.0545x over previous
"""ChunkEmbedding Trainium2 kernel.

Computation (see reference):
  chunk[n, :] = sum_l (w[n,l]/sum_l w[n,l]) * emb_table[input_ids[n,l], :]
  then scatter chunk rows into [B, T, D] at (map_ids[n], pos[n]+1), add
  CLS/SEP rows and build the mask.

Strategy:
  - Data-parallel over chunks: 4096 chunks -> 512 per core on 8 cores.
  - Per core: for each tile of 128 chunks, for each token l in 0..63,
    indirect-DMA-gather emb rows [128, 768] (partition = chunk) and
    fused multiply-accumulate on the vector engine:
        acc = x * w[:, l] + acc        (w is a per-partition scalar)
  - Weight normalization, final scatter into the padded [B, T, D] output,
    CLS/SEP and mask are trivial O(input/output-size) host-side prep on
    numpy (part of shard/unshard).
"""

import os

import numpy as np

N_CORES = 8
N_CHUNKS = 4096
CHUNK_LEN = 64
DIM = 768
VOCAB = 30522
CPC = N_CHUNKS // N_CORES  # chunks per core = 512
P = 128
N_TILES = CPC // P  # 4
CLS_IDX, SEP_IDX = 101, 102

# Pool depth for in-flight gather tiles (each is [128, G, 768] f32 = G*3KB/partition)
GATHER_BUFS = int(os.environ.get("CHUNKEMB_GATHER_BUFS", "6"))

_cache = {}


def _ensure_axon_ntff_hook():
    """The image's antenv package lacks axon_hooks, so trn_boot's NTFF
    profile hook registration silently degraded. Synthesize the module and
    register the ctypes-based hook so run_bass_kernel_spmd(trace=True) can
    capture NTFF profiles through the axon terminal."""
    import sys
    import types

    try:
        from antenv.axon_hooks import get_axon_ntff_profile_hook  # noqa: F401

        return True
    except ImportError:
        pass
    try:
        import antenv
        from trn_agent_boot.trn_boot import _ntff_profile_via_ctypes

        hook = _ntff_profile_via_ctypes("/opt/axon/libaxon_pjrt.so")
        if hook is None:
            return False
        mod = types.ModuleType("antenv.axon_hooks")
        mod._hook = hook
        mod.set_axon_ntff_profile_hook = lambda h: setattr(mod, "_hook", h)
        mod.get_axon_ntff_profile_hook = lambda: mod._hook
        sys.modules["antenv.axon_hooks"] = mod
        antenv.axon_hooks = mod
        return True
    except Exception:
        return False


# tokens gathered per dma_gather instruction (num_idxs = G*128)
G = int(os.environ.get("CHUNKEMB_G", "8"))
N_GROUPS = CHUNK_LEN // G
# "dve":   fused multiply-accumulate chain on VectorE
# "pe":    diagonal-weight matmuls accumulating in PSUM on TensorE
# "split": even tokens on VectorE chain, odd tokens on TensorE (diags via ScalarE)
IMPL = os.environ.get("CHUNKEMB_IMPL", "dve")
N_QUEUES = int(os.environ.get("CHUNKEMB_QUEUES", "1"))


def _build_program():
    import concourse.bacc as bacc
    import concourse.mybir as mybir
    import concourse.tile as tile

    nc = bacc.Bacc(
        "TRN2",
        target_bir_lowering=False,
        debug=False,
        num_devices=N_CORES,
        num_swdge_queues=N_QUEUES,
        dynamic_dma_scratch_size=int(
            os.environ.get("CHUNKEMB_SCRATCH", "16384")
        ),
    )
    emb = nc.dram_tensor(
        "emb", [VOCAB, DIM], mybir.dt.float32, kind="ExternalInput"
    ).ap()
    # int16 gather indices, pre-wrapped in 16 partitions (replicated to 128)
    # per dma_gather's layout: flat idx j lives at [j%16, j//16].
    idxs16 = nc.dram_tensor(
        "idxs16",
        [N_TILES, N_GROUPS, P, (G * P) // 16],
        mybir.dt.int16,
        kind="ExternalInput",
    ).ap()
    wn = nc.dram_tensor(
        "wn", [CPC, CHUNK_LEN], mybir.dt.float32, kind="ExternalInput"
    ).ap()
    out = nc.dram_tensor(
        "out", [CPC, DIM], mybir.dt.float32, kind="ExternalOutput"
    ).ap()

    num_idxs = G * P

    from concourse.masks import make_identity

    with tile.TileContext(nc) as tc:
        with (
            tc.tile_pool(name="xpool", bufs=GATHER_BUFS) as xpool,
            tc.tile_pool(name="idxp", bufs=4) as idxpool,
            tc.tile_pool(name="meta", bufs=2) as metapool,
            tc.tile_pool(name="accp", bufs=2) as accpool,
            tc.tile_pool(name="ident", bufs=1) as identpool,
            tc.tile_pool(name="diagp", bufs=4) as diagpool,
            tc.tile_pool(name="psump", bufs=2, space="PSUM") as psumpool,
        ):
            ident = None
            if IMPL in ("pe", "split"):
                ident = identpool.tile([P, P], mybir.dt.float32, tag="ident")
                make_identity(nc, ident[:])
            gather_i = 0
            for t in range(N_TILES):
                wn_t = metapool.tile([P, CHUNK_LEN], mybir.dt.float32, tag="wn")
                nc.sync.dma_start(out=wn_t[:], in_=wn[t * P : (t + 1) * P, :])
                psum_a = psum_b = None
                if IMPL in ("pe", "split"):
                    psum_a = psumpool.tile([P, 512], mybir.dt.float32, tag="pa")
                    psum_b = psumpool.tile([P, 256], mybir.dt.float32, tag="pb")
                acc = accpool.tile([P, DIM], mybir.dt.float32, tag="acc")
                # second independent DVE chain (odd gather groups) — halves the
                # serial accumulation latency per chunk-tile so gather buffer
                # slots recycle faster
                acc2 = None
                if IMPL == "dve":
                    acc2 = accpool.tile([P, DIM], mybir.dt.float32, tag="acc2")
                for gi in range(N_GROUPS):
                    idx_t = idxpool.tile(
                        [P, (G * P) // 16], mybir.dt.int16, tag="idx"
                    )
                    nc.sync.dma_start(out=idx_t[:], in_=idxs16[t, gi])
                    x = xpool.tile([P, G, DIM], mybir.dt.float32, tag="x")
                    nc.gpsimd.dma_gather(
                        x[:],
                        emb[:],
                        idx_t[:],
                        num_idxs,
                        num_idxs,
                        DIM,
                        queue_num=gather_i % N_QUEUES,
                    )
                    gather_i += 1
                    for g in range(G):
                        l = gi * G + g
                        on_pe = IMPL == "pe" or (IMPL == "split" and l % 2 == 1)
                        if on_pe:
                            diag = diagpool.tile(
                                [P, P], mybir.dt.float32, tag="diag"
                            )
                            if IMPL == "split":
                                # build diag on the otherwise-idle ScalarE
                                nc.scalar.activation(
                                    diag[:],
                                    ident[:],
                                    mybir.ActivationFunctionType.Copy,
                                    scale=wn_t[:, l : l + 1],
                                )
                                first = l == 1
                            else:
                                nc.vector.tensor_scalar_mul(
                                    diag[:], ident[:], wn_t[:, l : l + 1]
                                )
                                first = l == 0
                            nc.tensor.matmul(
                                out=psum_a[:],
                                lhsT=diag[:],
                                rhs=x[:, g, 0:512],
                                start=first,
                                stop=(l == CHUNK_LEN - 1),
                            )
                            nc.tensor.matmul(
                                out=psum_b[:],
                                lhsT=diag[:],
                                rhs=x[:, g, 512:DIM],
                                start=first,
                                stop=(l == CHUNK_LEN - 1),
                            )
                        else:
                            tgt = acc
                            chain_first = l == 0
                            if IMPL == "dve":
                                tgt = acc if gi % 2 == 0 else acc2
                                chain_first = gi in (0, 1) and g == 0
                            if chain_first:
                                nc.vector.tensor_scalar_mul(
                                    tgt[:], x[:, g, :], wn_t[:, l : l + 1]
                                )
                            else:
                                nc.vector.scalar_tensor_tensor(
                                    out=tgt[:],
                                    in0=x[:, g, :],
                                    scalar=wn_t[:, l : l + 1],
                                    in1=tgt[:],
                                    op0=mybir.AluOpType.mult,
                                    op1=mybir.AluOpType.add,
                                )
                if IMPL == "pe":
                    nc.scalar.copy(acc[:, 0:512], psum_a[:])
                    nc.scalar.copy(acc[:, 512:DIM], psum_b[:])
                elif IMPL == "split":
                    nc.vector.tensor_add(acc[:, 0:512], acc[:, 0:512], psum_a[:])
                    nc.vector.tensor_add(
                        acc[:, 512:DIM], acc[:, 512:DIM], psum_b[:]
                    )
                elif IMPL == "dve":
                    nc.vector.tensor_add(acc[:], acc[:], acc2[:])
                nc.sync.dma_start(out=out[t * P : (t + 1) * P, :], in_=acc[:])
    nc.compile()
    return nc


def _pack_idxs(ids_core):
    """ids_core [CPC, CHUNK_LEN] int -> [N_TILES, N_GROUPS, 128, G*128//16]
    int16 in dma_gather's wrapped layout: per (tile, group), flat position
    i = g*128 + c holds ids[tile*128+c, gi*G+g]; wrapped[p, s] = flat[s*16+p],
    replicated across the 8 groups of 16 partitions."""
    a = ids_core.reshape(N_TILES, P, N_GROUPS, G)  # t, c, gi, g
    a = a.transpose(0, 2, 3, 1)  # t, gi, g, c  -> flat (g*128+c)
    flat = a.reshape(N_TILES, N_GROUPS, G * P)
    wrapped = flat.reshape(N_TILES, N_GROUPS, (G * P) // 16, 16).transpose(
        0, 1, 3, 2
    )  # [t, gi, 16, s]
    rep = np.tile(wrapped, (1, 1, 8, 1))  # replicate to 128 partitions
    return np.ascontiguousarray(rep.astype(np.int16))


def _get_program():
    if "nc" not in _cache:
        _cache["nc"] = _build_program()
    return _cache["nc"]


def kernel(input_ids, kp_token_weights, map_ids, emb_table, batch_size, max_map_len):
    from concourse.bass_utils import run_bass_kernel_spmd

    ids = np.ascontiguousarray(np.asarray(input_ids, dtype=np.int32))
    w = np.asarray(kp_token_weights, dtype=np.float32)
    wn = np.ascontiguousarray(w / w.sum(axis=1, keepdims=True))
    emb = np.ascontiguousarray(np.asarray(emb_table, dtype=np.float32))
    assert ids.shape == (N_CHUNKS, CHUNK_LEN) and emb.shape == (VOCAB, DIM)

    nc = _get_program()
    in_maps = [
        {
            "emb": emb,
            "idxs16": _pack_idxs(ids[k * CPC : (k + 1) * CPC]),
            "wn": wn[k * CPC : (k + 1) * CPC],
        }
        for k in range(N_CORES)
    ]
    trace = bool(int(os.environ.get("CHUNKEMB_TRACE", "0")))
    if trace:
        trace = _ensure_axon_ntff_hook()
    res = run_bass_kernel_spmd(
        nc,
        in_maps,
        core_ids=list(range(N_CORES)),
        trace=trace,
        trace_cores=list(range(N_CORES)) if trace else None,
    )
    _cache["last_results"] = res
    chunk = np.concatenate([r["out"] for r in res.results], axis=0)  # [4096, 768]

    # Host-side unshard: scatter chunk rows into the padded output.
    mi = np.asarray(map_ids, dtype=np.int64)
    B = int(batch_size)
    T = int(max_map_len) + 2
    counts = np.bincount(mi, minlength=B)
    offsets = np.cumsum(counts) - counts
    pos = np.arange(mi.shape[0], dtype=np.int64) - offsets[mi]
    ret = np.zeros((B, T, DIM), dtype=np.float32)
    ret[mi, pos + 1] = chunk
    ret[:, 0, :] = emb[CLS_IDX]
    ret[np.arange(B), counts + 1] = emb[SEP_IDX]
    mask = np.zeros((B, T), dtype=np.float32)
    mask[mi, pos + 1] = 1.0
    mask[:, 0] = 1.0
    mask[np.arange(B), counts + 1] = 1.0
    return ret, mask
